# revision 20
# baseline (speedup 1.0000x reference)
"""Trainium2 Bass kernel for a top-2 MoE layer (T=2048, H=2048, I=1408, E=8).

Strategy: expert-parallel over 8 NeuronCores. The host dispatches tokens:
for each expert e it gathers the tokens routed to e (~480 of 2048, padded
to a shared capacity C sized to the busiest expert), so each core runs a
dense [C,H]x[2I,H]->silu*mul->[C,H] FFN for its expert — a 4x FLOP saving
over dense all-experts compute. The host then combines per-expert outputs
with the routing weights.

Device kernel (per core), transposed layout so no on-device transposes:
  stage 1: guT[2816, C] = w13 @ xT         (22 x 16 matmuls, K-tiles of 128)
  stage 2: actT[1408, C] = silu(gT) * uT   (ScalarE Silu + VectorE mul)
  stage 3: yT[2048, C] = w2 @ actT         (16 x 11 matmuls)
Matmuls in fp16 (fp32 accumulation in PSUM; rel-err ~5e-4).

Schedule details (v2):
 - ~20 warmup matmuls on a memset tile run while the first input DMAs are
   in flight, so the PE's HAM clock gate is already at 8/8 when the real
   stream starts (cold matmuls run at 1.2GHz vs 2.4GHz warm).
 - First-needed transfers (x chunk 0, w13[m=0] gate half) are triggered
   first, split across the two HWDGE trigger engines (Sync + Scalar) so
   trigger serialization doesn't delay the first matmul.
 - Few, large DMA transfers (w13 g+u halves fused per m-block, w2 in 4
   super-tiles, x in 4 chunks, y in pairs) cut trigger overhead and
   semaphore count.
 - Stage-1-only tile pools close before stage 3 is traced, so their
   release bookkeeping is scheduled under the stage-3 matmul stream
   instead of the end-of-kernel barrier.
 - The last yT row block is computed as two half-width matmul chains so
   its PSUM->SBUF copy + output DMA tail is halved.
"""

import sys

if "/opt/trn_rl_repo" not in sys.path:
    sys.path.insert(0, "/opt/trn_rl_repo")

import os
import numpy as np
from contextlib import ExitStack

import concourse.bass as bass
import concourse.tile as tile
from concourse import bacc, mybir

T, H, I, E, K = 2048, 2048, 1408, 8, 2
CMAX = 512                   # max token capacity per expert per pass (PSUM bank)
HT = H // 128                # 16 K-tiles over H
IT = I // 128                # 11 K-tiles over I
BT = 2 * I // 128            # 22 row-blocks of guT

import ml_dtypes

MODE = os.environ.get("KERNEL_DTYPE", "f16")
if MODE == "bf16":
    DT = mybir.dt.bfloat16
    NP_DT = ml_dtypes.bfloat16
elif MODE == "f16":
    DT = mybir.dt.float16
    NP_DT = np.float16
else:
    DT = mybir.dt.float32r
    NP_DT = np.float32

# Warmup matmuls bridge the initial DMA wait (~6.5us -> ~13us) keeping the
# PE's HAM clock-gate warm; the measured exec window opens at the framework
# preamble regardless, so these are free for the metric.
N_WARM = int(os.environ.get("KERNEL_WARMUP_MMS", "36"))

_cache: dict = {}


def _build_nc(C):
    """Build + compile the per-core FFN program (same program on all cores)."""
    nc = bacc.Bacc("TRN2", target_bir_lowering=False, debug=False, num_devices=E)
    # x in 4 chunks of 4 k-tiles: chunk j, cols kk*C:(kk+1)*C = k-tile 4j+kk
    x_d = nc.dram_tensor("x_sb", [4, 128, 4 * C], DT, kind="ExternalInput")
    # w13 m=0..2 as separate gate/up halves so early transfers are small and
    # can be staggered to match the matmul stream's consumption order
    w13a_d = nc.dram_tensor("w13a_sb", [6, 128, HT * 128], DT, kind="ExternalInput")
    # w13 m=3..10 fused gate+up: [128, 2*HT*128] each
    w13b_d = nc.dram_tensor(
        "w13b_sb", [IT - 3, 128, 2 * HT * 128], DT, kind="ExternalInput"
    )
    # w2 in 4 super-tiles of 4 m-blocks: cols ((m%4)*IT + k)*128
    w2_d = nc.dram_tensor("w2_sb", [4, 128, 4 * IT * 128], DT, kind="ExternalInput")
    # y out in fp16 (quantization ~2e-4 rel, well under budget; halves the
    # output DMA bytes and the end-of-kernel queue drain): 7 pairs + 2 singles
    y_d = nc.dram_tensor("y_sb", [7, 128, 2 * C], DT, kind="ExternalOutput")
    y2_d = nc.dram_tensor("y2_sb", [2, 128, C], DT, kind="ExternalOutput")

    AF = mybir.ActivationFunctionType
    F32 = mybir.dt.float32
    CL = C // 2          # last-block split: first half columns
    CR = C - CL

    with tile.TileContext(nc) as tc, ExitStack() as ctx:
        # ---- warmup: keep the PE busy (and HAM warm) while inputs land ----
        with tc.tile_pool(name="warm", bufs=1) as wrm, tc.tile_pool(
            name="warmp", bufs=1, space=bass.MemorySpace.PSUM
        ) as wrmp:
            wt = wrm.tile([128, 256], DT, tag="wt")
            nc.gpsimd.memset(wt[:], 0.0)
            wps = wrmp.tile([128, 256], F32, tag="wps")
            for _ in range(N_WARM):
                nc.tensor.matmul(wps[:], wt[:, 0:128], wt[:], start=True, stop=True)

        # ---- stage 1+2 pools (closed before stage 3 so releases hide) ----
        act_t = []
        ap = ctx.enter_context(tc.tile_pool(name="act", bufs=1))
        wp2 = ctx.enter_context(tc.tile_pool(name="w2", bufs=3))
        # psy opened BEFORE psg so stage-3 PSUM tiles don't wait on the
        # stage-1 bank release (psg 5 + psy 3 = 8 banks coexist)
        psy = ctx.enter_context(
            tc.tile_pool(name="psy", bufs=3, space=bass.MemorySpace.PSUM)
        )
        with tc.tile_pool(name="x", bufs=1) as xp, tc.tile_pool(
            name="w13a", bufs=6
        ) as wpa, tc.tile_pool(name="w13b", bufs=3) as wpb, tc.tile_pool(
            name="tmp1", bufs=2
        ) as sp1, tc.tile_pool(
            name="psg", bufs=5, space=bass.MemorySpace.PSUM
        ) as psg:
            # Head transfers, staggered in consumption order. DMA queues
            # drain roughly FIFO per trigger engine, so issue order ==
            # arrival order; small first transfers start the stream early.
            x_t = []
            xc = []
            for j in range(4):
                xt = xp.tile([128, 4 * C], DT, tag=f"x{j}", name=f"x{j}")
                xc.append(xt)
            wa = [
                wpa.tile([128, HT * 128], DT, tag="w13a", name=f"w13a{i}")
                for i in range(6)
            ]  # g0,u0,g1,u1,g2,u2
            HB = HT * 128 // 2
            # sync engine: x chunks + m1 halves
            nc.sync.dma_start(xc[0][:], x_d.ap()[0])
            nc.sync.dma_start(xc[1][:], x_d.ap()[1])
            nc.sync.dma_start(xc[2][:], x_d.ap()[2])
            # scalar engine: m0 weight halves (quarter transfers) in the
            # order the interleaved g/u chains consume them, then x chunk 3
            nc.scalar.dma_start(wa[0][:, 0:HB], w13a_d.ap()[0][:, 0:HB])
            nc.scalar.dma_start(wa[1][:, 0:HB], w13a_d.ap()[1][:, 0:HB])
            nc.scalar.dma_start(wa[0][:, HB:], w13a_d.ap()[0][:, HB:])
            nc.scalar.dma_start(wa[1][:, HB:], w13a_d.ap()[1][:, HB:])
            nc.scalar.dma_start(xc[3][:], x_d.ap()[3])
            # m1 / m2 gate+up halves
            nc.sync.dma_start(wa[2][:], w13a_d.ap()[2])
            nc.sync.dma_start(wa[3][:], w13a_d.ap()[3])
            nc.scalar.dma_start(wa[4][:], w13a_d.ap()[4])
            nc.scalar.dma_start(wa[5][:], w13a_d.ap()[5])
            for j in range(4):
                for kk in range(4):
                    x_t.append(xc[j][:, kk * C : (kk + 1) * C])

            wgu = {}

            def _load_w13(m, eng):
                # m >= 3
                wb = wpb.tile([128, 2 * HT * 128], DT, tag="w13b", name=f"wgu{m}")
                eng.dma_start(wb[:], w13b_d.ap()[m - 3])
                wgu[m] = wb

            w2t = []

            def _load_w2(q):
                wt2 = wp2.tile([128, 4 * IT * 128], DT, tag="w2", name=f"w2q{q}")
                nc.sync.dma_start(wt2[:], w2_d.ap()[q])
                w2t.append(wt2)

            # stage 1+2: guT blocks
            for m in range(IT):
                if m < 3:
                    g_w = [wa[2 * m][:, k * 128 : (k + 1) * 128] for k in range(HT)]
                    u_w = [
                        wa[2 * m + 1][:, k * 128 : (k + 1) * 128] for k in range(HT)
                    ]
                else:
                    if m not in wgu:
                        _load_w13(m, nc.sync)
                    wb = wgu.pop(m)
                    g_w = [wb[:, k * 128 : (k + 1) * 128] for k in range(HT)]
                    u_w = [
                        wb[:, (HT + k) * 128 : (HT + k + 1) * 128] for k in range(HT)
                    ]
                g_ps = psg.tile([128, C], F32, tag="ps")
                u_ps = psg.tile([128, C], F32, tag="ps")
                if m == 0:
                    # interleave g/u in blocks of 4 k-tiles: halves the x
                    # delivery rate the first chains demand while the head
                    # DMA burst is still in flight
                    for kb in range(0, HT, 4):
                        for k in range(kb, kb + 4):
                            nc.tensor.matmul(
                                g_ps[:], g_w[k], x_t[k],
                                start=(k == 0), stop=(k == HT - 1),
                            )
                        for k in range(kb, kb + 4):
                            nc.tensor.matmul(
                                u_ps[:], u_w[k], x_t[k],
                                start=(k == 0), stop=(k == HT - 1),
                            )
                else:
                    for k in range(HT):
                        nc.tensor.matmul(
                            g_ps[:], g_w[k], x_t[k],
                            start=(k == 0), stop=(k == HT - 1),
                        )
                    for k in range(HT):
                        nc.tensor.matmul(
                            u_ps[:], u_w[k], x_t[k],
                            start=(k == 0), stop=(k == HT - 1),
                        )
                sg = sp1.tile([128, C], F32, tag="sg")
                nc.scalar.activation(sg[:], g_ps[:], AF.Silu)
                at = ap.tile([128, C], DT, tag=f"act{m}")
                nc.vector.tensor_mul(at[:], sg[:], u_ps[:])
                act_t.append(at)
                # prefetch: next-next w13 block + w2 super-tiles mid-stream
                if m + 2 < IT and m + 2 >= 3 and (m + 2) not in wgu:
                    _load_w13(m + 2, nc.sync if m % 2 else nc.scalar)
                if m == 4:
                    _load_w2(0)
                elif m == 6:
                    _load_w2(1)
                elif m == 8:
                    _load_w2(2)
                elif m == 10:
                    _load_w2(3)

        # ---- stage 3: yT row-blocks ----
        sp2 = ctx.enter_context(tc.tile_pool(name="tmp2", bufs=2))
        # half-width PSUM tiles for the split last chain; opened after psg
        # closed (reuses freed banks, long before they're needed)
        psyh = ctx.enter_context(
            tc.tile_pool(name="psyh", bufs=2, space=bass.MemorySpace.PSUM)
        )
        ysb = []
        for m in range(HT):
            qq = m // 4
            w2tile = w2t[qq]
            coff = (m % 4) * IT * 128

            if m < 14:
                if m % 2 == 0:
                    yt = sp2.tile([128, 2 * C], DT, tag="yout2")
                    ysb.append(yt)
                y_sb = ysb[m // 2]
                dst = y_sb[:, (m % 2) * C : (m % 2 + 1) * C]
            else:
                yt = sp2.tile([128, C], DT, tag="yout1")
                ysb.append(yt)
                y_sb = yt
                dst = y_sb[:]

            if m == HT - 1:
                # split the final chain so the tail drain is half-width,
                # and DMA each half out as soon as its copy lands
                y_psL = psyh.tile([128, CL], F32, tag="yh")
                y_psR = psyh.tile([128, CR], F32, tag="yh")
                for k in range(IT):
                    wsl = w2tile[:, coff + k * 128 : coff + (k + 1) * 128]
                    nc.tensor.matmul(
                        y_psL[:], wsl, act_t[k][:, 0:CL],
                        start=(k == 0), stop=(k == IT - 1),
                    )
                nc.scalar.copy(y_sb[:, 0:CL], y_psL[:])
                nc.sync.dma_start(y2_d.ap()[1][:, 0:CL], y_sb[:, 0:CL])
                for k in range(IT):
                    wsl = w2tile[:, coff + k * 128 : coff + (k + 1) * 128]
                    nc.tensor.matmul(
                        y_psR[:], wsl, act_t[k][:, CL:C],
                        start=(k == 0), stop=(k == IT - 1),
                    )
                nc.scalar.copy(y_sb[:, CL:C], y_psR[:])
                nc.sync.dma_start(y2_d.ap()[1][:, CL:C], y_sb[:, CL:C])
            else:
                y_ps = psy.tile([128, C], F32, tag="y")
                for k in range(IT):
                    nc.tensor.matmul(
                        y_ps[:], w2tile[:, coff + k * 128 : coff + (k + 1) * 128],
                        act_t[k][:],
                        start=(k == 0), stop=(k == IT - 1),
                    )
                nc.scalar.copy(dst, y_ps[:])
                if m < 14 and m % 2 == 1:
                    nc.sync.dma_start(y_d.ap()[m // 2], y_sb[:])
                elif m == 14:
                    nc.sync.dma_start(y2_d.ap()[0], y_sb[:])

    nc.compile()
    return nc


def _get_nc(C):
    if C not in _cache:
        _cache[C] = _build_nc(C)
    return _cache[C]


def _prep_weights(w13, w2):
    """Pre-tile weights into the SBUF layouts the kernel DMAs verbatim.

    w13 k-tile layout per row-block b (of 22): [128 rows, HT*128 cols] where
    col k*128+p holds w13[e, b*128+row, k*128+p] transposed so the 128
    partition dim is the contraction dim:
      w13_blk[e, b, p, k*128+c] = w13[e, b*128+c, k*128+p]
    Gate block m pairs with up block m+IT.
      w13a: m=0 gate, m=0 up (two separate transfers)
      w13b: m=1..10, gate||up fused into [128, 2*HT*128]
    w2 super-tile q holds m-blocks 4q..4q+3:
      w2_sb[e, q, p, ((m%4)*IT+k)*128+c] = w2[e, m*128+c, k*128+p]
    """
    w13_blk = (
        w13.reshape(E, BT, 128, HT, 128)
        .transpose(0, 1, 4, 3, 2)
        .astype(NP_DT)
        .reshape(E, BT, 128, HT * 128)
    )
    # m=0..2 as separate gate/up halves: [g0,u0,g1,u1,g2,u2]
    w13a = np.ascontiguousarray(
        w13_blk[:, [0, IT, 1, IT + 1, 2, IT + 2]]
    )  # [E, 6, 128, HT*128]
    gu = np.stack([w13_blk[:, 3:IT], w13_blk[:, IT + 3 :]], axis=2)
    # gu: [E, 8, 2, 128, HT*128] -> fuse (2, HT*128) cols per partition
    w13b = np.ascontiguousarray(gu.transpose(0, 1, 3, 2, 4)).reshape(
        E, IT - 3, 128, 2 * HT * 128
    )
    w2_blk = (
        w2.reshape(E, HT, 128, IT, 128)
        .transpose(0, 1, 4, 3, 2)
        .astype(NP_DT)
        .reshape(E, HT, 128, IT * 128)
    )  # [E, m, 128, IT*128]
    w2_sb = np.ascontiguousarray(
        w2_blk.reshape(E, 4, 4, 128, IT * 128).transpose(0, 1, 3, 2, 4)
    ).reshape(E, 4, 128, 4 * IT * 128)
    return w13a, w13b, w2_sb


def kernel(
    hidden_states,
    topk_weights,
    topk_ids,
    w13,
    w2,
    num_global_tokens=None,
    max_num_tokens_per_gpu=None,
):
    from concourse.bass_utils import run_bass_kernel_spmd

    hs = np.asarray(hidden_states, dtype=np.float32)
    tw = np.asarray(topk_weights, dtype=np.float32)
    ti = np.asarray(topk_ids)
    w13 = np.asarray(w13, dtype=np.float32)
    w2 = np.asarray(w2, dtype=np.float32)

    assert hs.shape == (T, H), hs.shape
    assert w13.shape == (E, 2 * I, H), w13.shape
    assert w2.shape == (E, H, I), w2.shape

    # per-(token, expert) combine weights: sum of topk weights routed to e
    comb = np.zeros((T, E), dtype=np.float32)
    for k in range(ti.shape[1]):
        col = ti[:, k]
        ok = (col >= 0) & (col < E)
        np.add.at(comb, (np.arange(T)[ok], col[ok]), tw[ok, k])

    idxs = [np.nonzero(comb[:, e])[0] for e in range(E)]
    need = max(len(ix) for ix in idxs)
    C = min(CMAX, max(64, need))
    if C % 2:
        C += 1  # even so the final split chain halves cleanly
    nchunks = max(1, -(-need // C))

    w13a, w13b, w2_sb = _prep_weights(w13, w2)
    nc = _get_nc(C)

    trace = bool(os.environ.get("KERNEL_PROFILE"))
    out = np.zeros((T, H), dtype=np.float32)
    for chunk in range(nchunks):
        in_maps = []
        sels = []
        for e in range(E):
            sel = idxs[e][chunk * C : (chunk + 1) * C]
            xe = np.zeros((C, H), dtype=np.float32)
            xe[: len(sel)] = hs[sel]
            # x chunks: [4, 128, 4*C]; chunk j col block kk = k-tile 4j+kk
            x_sb = (
                np.ascontiguousarray(
                    xe.T.reshape(4, 4, 128, C).transpose(0, 2, 1, 3)
                )
                .astype(NP_DT, copy=False)
                .reshape(4, 128, 4 * C)
            )
            in_maps.append(
                {
                    "x_sb": x_sb,
                    "w13a_sb": w13a[e],
                    "w13b_sb": w13b[e],
                    "w2_sb": w2_sb[e],
                }
            )
            sels.append(sel)
        if trace:
            try:
                res = run_bass_kernel_spmd(nc, in_maps, list(range(E)), trace=True)
                if res.exec_time_ns is not None:
                    print(f"HW exec time: {res.exec_time_ns} ns")
            except Exception:
                res = run_bass_kernel_spmd(nc, in_maps, list(range(E)))
        else:
            res = run_bass_kernel_spmd(nc, in_maps, list(range(E)))
        for e in range(E):
            sel = sels[e]
            if len(sel) == 0:
                continue
            y_pair = np.asarray(res.results[e]["y_sb"], dtype=np.float32)
            y_sing = np.asarray(res.results[e]["y2_sb"], dtype=np.float32)
            # decode: pairs hold m=2j (cols 0:C) and m=2j+1 (cols C:2C)
            ye = np.empty((H, C), dtype=np.float32)
            yp = y_pair.reshape(7, 128, 2, C).transpose(0, 2, 1, 3).reshape(14 * 128, C)
            ye[: 14 * 128] = yp
            ye[14 * 128 : 15 * 128] = y_sing[0]
            ye[15 * 128 :] = y_sing[1]
            ye = ye.T  # [C, H]
            out[sel] += comb[sel, e][:, None] * ye[: len(sel)]
    return out


# revision 21
# speedup vs baseline: 1.1767x; 1.1767x over previous
"""Trainium2 Bass kernel for a top-2 MoE layer (T=2048, H=2048, I=1408, E=8).

Strategy: expert-parallel over 8 NeuronCores. The host dispatches tokens:
for each expert e it gathers the tokens routed to e (~480 of 2048, padded
to a shared capacity C sized to the busiest expert), so each core runs a
dense [C,H]x[2I,H]->silu*mul->[C,H] FFN for its expert — a 4x FLOP saving
over dense all-experts compute. The host then combines per-expert outputs
with the routing weights.

Device kernel (per core), transposed layout so no on-device transposes:
  stage 1: guT[2816, C] = w13 @ xT         (22 x 16 matmuls, K-tiles of 128)
  stage 2: actT[1408, C] = silu(gT) * uT   (ScalarE Silu + VectorE mul)
  stage 3: yT[2048, C] = w2 @ actT         (16 x 11 matmuls)
Matmuls in fp16 (fp32 accumulation in PSUM; rel-err ~5e-4).

Schedule details:
 - 36 warmup matmuls on a memset scratch tile bridge the initial input-DMA
   wait (~6.5us -> ~13.5us), so the PE's HAM clock gate is already 8/8
   when the real stream starts (cold matmuls run at half clock). The
   measured exec window opens at the framework preamble either way, so
   the warmups are free; they must end close to data arrival or a >3.4us
   PE idle re-throttles the clock gate.
 - Head transfers are staggered in consumption order across the two HWDGE
   trigger engines (Sync: x chunks, Scalar: m0 weight quarters) and the
   m=0 gate/up chains are interleaved in blocks of 4 k-tiles, halving the
   x-bandwidth the first chains demand while the 3MB head burst (shared
   HBM, all 8 cores at once) is still in flight.
 - Few, large DMA transfers (w13 g+u fused per m-block for m>=3, w2 in 4
   super-tiles, x in 4 chunks, y in pairs) cut trigger overhead.
 - psy (stage-3 PSUM) is allocated alongside psg (8 banks total) so the
   first stage-3 matmul doesn't wait on the stage-1 bank release; w2 gets
   3 buffers so its 3rd super-tile load isn't gated by a tile release.
 - Stage-1-only tile pools close before stage 3 is traced, so their
   release bookkeeping hides under the stage-3 matmul stream.
 - y returns in fp16 (adds ~2e-4 rel error, halves output DMA) and the
   last yT row block is computed as two half-width chains, each half
   DMA'd as soon as its copy lands -- the end-of-kernel queue drain that
   the measured window includes is halved.

Measured (8 cores, all-core NTFF profiling, max over cores): ~134.0us in
the chip's 2.4GHz PE state vs 134.7us for the previous baseline; the
matmul stream itself runs gapless at N/2.4GHz+2.5ns per matmul. Note the
shared trn2 package sometimes sits in a ~2.0GHz throttled state (253ns
per matmul) where any config measures ~156-158us; comparisons are only
valid within the same clock regime.
"""

import sys

if "/opt/trn_rl_repo" not in sys.path:
    sys.path.insert(0, "/opt/trn_rl_repo")

import os
import numpy as np
from contextlib import ExitStack

import concourse.bass as bass
import concourse.tile as tile
from concourse import bacc, mybir

T, H, I, E, K = 2048, 2048, 1408, 8, 2
CMAX = 512                   # max token capacity per expert per pass (PSUM bank)
HT = H // 128                # 16 K-tiles over H
IT = I // 128                # 11 K-tiles over I
BT = 2 * I // 128            # 22 row-blocks of guT

import ml_dtypes

MODE = os.environ.get("KERNEL_DTYPE", "f16")
if MODE == "bf16":
    DT = mybir.dt.bfloat16
    NP_DT = ml_dtypes.bfloat16
elif MODE == "f16":
    DT = mybir.dt.float16
    NP_DT = np.float16
else:
    DT = mybir.dt.float32r
    NP_DT = np.float32

# Warmup matmuls bridge the initial DMA wait (~6.5us -> ~13us) keeping the
# PE's HAM clock-gate warm; the measured exec window opens at the framework
# preamble regardless, so these are free for the metric.
N_WARM = int(os.environ.get("KERNEL_WARMUP_MMS", "36"))

_cache: dict = {}


def _build_nc(C):
    """Build + compile the per-core FFN program (same program on all cores)."""
    nc = bacc.Bacc("TRN2", target_bir_lowering=False, debug=False, num_devices=E)
    # x in 4 chunks of 4 k-tiles: chunk j, cols kk*C:(kk+1)*C = k-tile 4j+kk
    x_d = nc.dram_tensor("x_sb", [4, 128, 4 * C], DT, kind="ExternalInput")
    # w13 m=0..2 as separate gate/up halves so early transfers are small and
    # can be staggered to match the matmul stream's consumption order
    w13a_d = nc.dram_tensor("w13a_sb", [6, 128, HT * 128], DT, kind="ExternalInput")
    # w13 m=3..10 fused gate+up: [128, 2*HT*128] each
    w13b_d = nc.dram_tensor(
        "w13b_sb", [IT - 3, 128, 2 * HT * 128], DT, kind="ExternalInput"
    )
    # w2 in 4 super-tiles of 4 m-blocks: cols ((m%4)*IT + k)*128
    w2_d = nc.dram_tensor("w2_sb", [4, 128, 4 * IT * 128], DT, kind="ExternalInput")
    # y out in fp16 (quantization ~2e-4 rel, well under budget; halves the
    # output DMA bytes and the end-of-kernel queue drain): 7 pairs + 2 singles
    y_d = nc.dram_tensor("y_sb", [7, 128, 2 * C], DT, kind="ExternalOutput")
    y2_d = nc.dram_tensor("y2_sb", [2, 128, C], DT, kind="ExternalOutput")

    AF = mybir.ActivationFunctionType
    F32 = mybir.dt.float32
    CL = C // 2          # last-block split: first half columns
    CR = C - CL

    with tile.TileContext(nc) as tc, ExitStack() as ctx:
        # ---- warmup: keep the PE busy (and HAM warm) while inputs land ----
        with tc.tile_pool(name="warm", bufs=1) as wrm, tc.tile_pool(
            name="warmp", bufs=1, space=bass.MemorySpace.PSUM
        ) as wrmp:
            wt = wrm.tile([128, 256], DT, tag="wt")
            nc.gpsimd.memset(wt[:], 0.0)
            wps = wrmp.tile([128, 256], F32, tag="wps")
            for _ in range(N_WARM):
                nc.tensor.matmul(wps[:], wt[:, 0:128], wt[:], start=True, stop=True)

        # ---- stage 1+2 pools (closed before stage 3 so releases hide) ----
        act_t = []
        ap = ctx.enter_context(tc.tile_pool(name="act", bufs=1))
        wp2 = ctx.enter_context(tc.tile_pool(name="w2", bufs=3))
        # psy opened BEFORE psg so stage-3 PSUM tiles don't wait on the
        # stage-1 bank release (psg 5 + psy 3 = 8 banks coexist)
        psy = ctx.enter_context(
            tc.tile_pool(name="psy", bufs=3, space=bass.MemorySpace.PSUM)
        )
        with tc.tile_pool(name="x", bufs=1) as xp, tc.tile_pool(
            name="w13a", bufs=6
        ) as wpa, tc.tile_pool(name="w13b", bufs=3) as wpb, tc.tile_pool(
            name="tmp1", bufs=2
        ) as sp1, tc.tile_pool(
            name="psg", bufs=5, space=bass.MemorySpace.PSUM
        ) as psg:
            # Head transfers, staggered in consumption order. DMA queues
            # drain roughly FIFO per trigger engine, so issue order ==
            # arrival order; small first transfers start the stream early.
            x_t = []
            xc = []
            for j in range(4):
                xt = xp.tile([128, 4 * C], DT, tag=f"x{j}", name=f"x{j}")
                xc.append(xt)
            wa = [
                wpa.tile([128, HT * 128], DT, tag="w13a", name=f"w13a{i}")
                for i in range(6)
            ]  # g0,u0,g1,u1,g2,u2
            HB = HT * 128 // 2
            # sync engine: x chunks + m1 halves
            nc.sync.dma_start(xc[0][:], x_d.ap()[0])
            nc.sync.dma_start(xc[1][:], x_d.ap()[1])
            nc.sync.dma_start(xc[2][:], x_d.ap()[2])
            # scalar engine: m0 weight halves (quarter transfers) in the
            # order the interleaved g/u chains consume them, then x chunk 3
            nc.scalar.dma_start(wa[0][:, 0:HB], w13a_d.ap()[0][:, 0:HB])
            nc.scalar.dma_start(wa[1][:, 0:HB], w13a_d.ap()[1][:, 0:HB])
            nc.scalar.dma_start(wa[0][:, HB:], w13a_d.ap()[0][:, HB:])
            nc.scalar.dma_start(wa[1][:, HB:], w13a_d.ap()[1][:, HB:])
            nc.scalar.dma_start(xc[3][:], x_d.ap()[3])
            # m1 / m2 gate+up halves
            nc.sync.dma_start(wa[2][:], w13a_d.ap()[2])
            nc.sync.dma_start(wa[3][:], w13a_d.ap()[3])
            nc.scalar.dma_start(wa[4][:], w13a_d.ap()[4])
            nc.scalar.dma_start(wa[5][:], w13a_d.ap()[5])
            for j in range(4):
                for kk in range(4):
                    x_t.append(xc[j][:, kk * C : (kk + 1) * C])

            wgu = {}

            def _load_w13(m, eng):
                # m >= 3
                wb = wpb.tile([128, 2 * HT * 128], DT, tag="w13b", name=f"wgu{m}")
                eng.dma_start(wb[:], w13b_d.ap()[m - 3])
                wgu[m] = wb

            w2t = []

            def _load_w2(q):
                wt2 = wp2.tile([128, 4 * IT * 128], DT, tag="w2", name=f"w2q{q}")
                nc.sync.dma_start(wt2[:], w2_d.ap()[q])
                w2t.append(wt2)

            # stage 1+2: guT blocks
            for m in range(IT):
                if m < 3:
                    g_w = [wa[2 * m][:, k * 128 : (k + 1) * 128] for k in range(HT)]
                    u_w = [
                        wa[2 * m + 1][:, k * 128 : (k + 1) * 128] for k in range(HT)
                    ]
                else:
                    if m not in wgu:
                        _load_w13(m, nc.sync)
                    wb = wgu.pop(m)
                    g_w = [wb[:, k * 128 : (k + 1) * 128] for k in range(HT)]
                    u_w = [
                        wb[:, (HT + k) * 128 : (HT + k + 1) * 128] for k in range(HT)
                    ]
                g_ps = psg.tile([128, C], F32, tag="ps")
                u_ps = psg.tile([128, C], F32, tag="ps")
                if m == 0:
                    # interleave g/u in blocks of 4 k-tiles: halves the x
                    # delivery rate the first chains demand while the head
                    # DMA burst is still in flight
                    for kb in range(0, HT, 4):
                        for k in range(kb, kb + 4):
                            nc.tensor.matmul(
                                g_ps[:], g_w[k], x_t[k],
                                start=(k == 0), stop=(k == HT - 1),
                            )
                        for k in range(kb, kb + 4):
                            nc.tensor.matmul(
                                u_ps[:], u_w[k], x_t[k],
                                start=(k == 0), stop=(k == HT - 1),
                            )
                else:
                    for k in range(HT):
                        nc.tensor.matmul(
                            g_ps[:], g_w[k], x_t[k],
                            start=(k == 0), stop=(k == HT - 1),
                        )
                    for k in range(HT):
                        nc.tensor.matmul(
                            u_ps[:], u_w[k], x_t[k],
                            start=(k == 0), stop=(k == HT - 1),
                        )
                sg = sp1.tile([128, C], F32, tag="sg")
                nc.scalar.activation(sg[:], g_ps[:], AF.Silu)
                at = ap.tile([128, C], DT, tag=f"act{m}")
                nc.vector.tensor_mul(at[:], sg[:], u_ps[:])
                act_t.append(at)
                # prefetch: next-next w13 block + w2 super-tiles mid-stream
                if m + 2 < IT and m + 2 >= 3 and (m + 2) not in wgu:
                    _load_w13(m + 2, nc.sync if m % 2 else nc.scalar)
                if m == 4:
                    _load_w2(0)
                elif m == 6:
                    _load_w2(1)
                elif m == 8:
                    _load_w2(2)
                elif m == 10:
                    _load_w2(3)

        # ---- stage 3: yT row-blocks ----
        sp2 = ctx.enter_context(tc.tile_pool(name="tmp2", bufs=2))
        # half-width PSUM tiles for the split last chain; opened after psg
        # closed (reuses freed banks, long before they're needed)
        psyh = ctx.enter_context(
            tc.tile_pool(name="psyh", bufs=2, space=bass.MemorySpace.PSUM)
        )
        ysb = []
        for m in range(HT):
            qq = m // 4
            w2tile = w2t[qq]
            coff = (m % 4) * IT * 128

            if m < 14:
                if m % 2 == 0:
                    yt = sp2.tile([128, 2 * C], DT, tag="yout2")
                    ysb.append(yt)
                y_sb = ysb[m // 2]
                dst = y_sb[:, (m % 2) * C : (m % 2 + 1) * C]
            else:
                yt = sp2.tile([128, C], DT, tag="yout1")
                ysb.append(yt)
                y_sb = yt
                dst = y_sb[:]

            if m == HT - 1:
                # split the final chain so the tail drain is half-width,
                # and DMA each half out as soon as its copy lands
                y_psL = psyh.tile([128, CL], F32, tag="yh")
                y_psR = psyh.tile([128, CR], F32, tag="yh")
                for k in range(IT):
                    wsl = w2tile[:, coff + k * 128 : coff + (k + 1) * 128]
                    nc.tensor.matmul(
                        y_psL[:], wsl, act_t[k][:, 0:CL],
                        start=(k == 0), stop=(k == IT - 1),
                    )
                nc.scalar.copy(y_sb[:, 0:CL], y_psL[:])
                nc.sync.dma_start(y2_d.ap()[1][:, 0:CL], y_sb[:, 0:CL])
                for k in range(IT):
                    wsl = w2tile[:, coff + k * 128 : coff + (k + 1) * 128]
                    nc.tensor.matmul(
                        y_psR[:], wsl, act_t[k][:, CL:C],
                        start=(k == 0), stop=(k == IT - 1),
                    )
                nc.scalar.copy(y_sb[:, CL:C], y_psR[:])
                nc.sync.dma_start(y2_d.ap()[1][:, CL:C], y_sb[:, CL:C])
            else:
                y_ps = psy.tile([128, C], F32, tag="y")
                for k in range(IT):
                    nc.tensor.matmul(
                        y_ps[:], w2tile[:, coff + k * 128 : coff + (k + 1) * 128],
                        act_t[k][:],
                        start=(k == 0), stop=(k == IT - 1),
                    )
                nc.scalar.copy(dst, y_ps[:])
                if m < 14 and m % 2 == 1:
                    nc.sync.dma_start(y_d.ap()[m // 2], y_sb[:])
                elif m == 14:
                    nc.sync.dma_start(y2_d.ap()[0], y_sb[:])

    nc.compile()
    return nc


def _get_nc(C):
    if C not in _cache:
        _cache[C] = _build_nc(C)
    return _cache[C]


def _prep_weights(w13, w2):
    """Pre-tile weights into the SBUF layouts the kernel DMAs verbatim.

    w13 k-tile layout per row-block b (of 22): [128 rows, HT*128 cols] where
    col k*128+p holds w13[e, b*128+row, k*128+p] transposed so the 128
    partition dim is the contraction dim:
      w13_blk[e, b, p, k*128+c] = w13[e, b*128+c, k*128+p]
    Gate block m pairs with up block m+IT.
      w13a: m=0 gate, m=0 up (two separate transfers)
      w13b: m=1..10, gate||up fused into [128, 2*HT*128]
    w2 super-tile q holds m-blocks 4q..4q+3:
      w2_sb[e, q, p, ((m%4)*IT+k)*128+c] = w2[e, m*128+c, k*128+p]
    """
    w13_blk = (
        w13.reshape(E, BT, 128, HT, 128)
        .transpose(0, 1, 4, 3, 2)
        .astype(NP_DT)
        .reshape(E, BT, 128, HT * 128)
    )
    # m=0..2 as separate gate/up halves: [g0,u0,g1,u1,g2,u2]
    w13a = np.ascontiguousarray(
        w13_blk[:, [0, IT, 1, IT + 1, 2, IT + 2]]
    )  # [E, 6, 128, HT*128]
    gu = np.stack([w13_blk[:, 3:IT], w13_blk[:, IT + 3 :]], axis=2)
    # gu: [E, 8, 2, 128, HT*128] -> fuse (2, HT*128) cols per partition
    w13b = np.ascontiguousarray(gu.transpose(0, 1, 3, 2, 4)).reshape(
        E, IT - 3, 128, 2 * HT * 128
    )
    w2_blk = (
        w2.reshape(E, HT, 128, IT, 128)
        .transpose(0, 1, 4, 3, 2)
        .astype(NP_DT)
        .reshape(E, HT, 128, IT * 128)
    )  # [E, m, 128, IT*128]
    w2_sb = np.ascontiguousarray(
        w2_blk.reshape(E, 4, 4, 128, IT * 128).transpose(0, 1, 3, 2, 4)
    ).reshape(E, 4, 128, 4 * IT * 128)
    return w13a, w13b, w2_sb


def kernel(
    hidden_states,
    topk_weights,
    topk_ids,
    w13,
    w2,
    num_global_tokens=None,
    max_num_tokens_per_gpu=None,
):
    from concourse.bass_utils import run_bass_kernel_spmd

    hs = np.asarray(hidden_states, dtype=np.float32)
    tw = np.asarray(topk_weights, dtype=np.float32)
    ti = np.asarray(topk_ids)
    w13 = np.asarray(w13, dtype=np.float32)
    w2 = np.asarray(w2, dtype=np.float32)

    assert hs.shape == (T, H), hs.shape
    assert w13.shape == (E, 2 * I, H), w13.shape
    assert w2.shape == (E, H, I), w2.shape

    # per-(token, expert) combine weights: sum of topk weights routed to e
    comb = np.zeros((T, E), dtype=np.float32)
    for k in range(ti.shape[1]):
        col = ti[:, k]
        ok = (col >= 0) & (col < E)
        np.add.at(comb, (np.arange(T)[ok], col[ok]), tw[ok, k])

    idxs = [np.nonzero(comb[:, e])[0] for e in range(E)]
    need = max(len(ix) for ix in idxs)
    C = min(CMAX, max(64, need))
    if C % 2:
        C += 1  # even so the final split chain halves cleanly
    nchunks = max(1, -(-need // C))

    w13a, w13b, w2_sb = _prep_weights(w13, w2)
    nc = _get_nc(C)

    trace = bool(os.environ.get("KERNEL_PROFILE"))
    out = np.zeros((T, H), dtype=np.float32)
    for chunk in range(nchunks):
        in_maps = []
        sels = []
        for e in range(E):
            sel = idxs[e][chunk * C : (chunk + 1) * C]
            xe = np.zeros((C, H), dtype=np.float32)
            xe[: len(sel)] = hs[sel]
            # x chunks: [4, 128, 4*C]; chunk j col block kk = k-tile 4j+kk
            x_sb = (
                np.ascontiguousarray(
                    xe.T.reshape(4, 4, 128, C).transpose(0, 2, 1, 3)
                )
                .astype(NP_DT, copy=False)
                .reshape(4, 128, 4 * C)
            )
            in_maps.append(
                {
                    "x_sb": x_sb,
                    "w13a_sb": w13a[e],
                    "w13b_sb": w13b[e],
                    "w2_sb": w2_sb[e],
                }
            )
            sels.append(sel)
        if trace:
            try:
                res = run_bass_kernel_spmd(nc, in_maps, list(range(E)), trace=True)
                if res.exec_time_ns is not None:
                    print(f"HW exec time: {res.exec_time_ns} ns")
            except Exception:
                res = run_bass_kernel_spmd(nc, in_maps, list(range(E)))
        else:
            res = run_bass_kernel_spmd(nc, in_maps, list(range(E)))
        for e in range(E):
            sel = sels[e]
            if len(sel) == 0:
                continue
            y_pair = np.asarray(res.results[e]["y_sb"], dtype=np.float32)
            y_sing = np.asarray(res.results[e]["y2_sb"], dtype=np.float32)
            # decode: pairs hold m=2j (cols 0:C) and m=2j+1 (cols C:2C)
            ye = np.empty((H, C), dtype=np.float32)
            yp = y_pair.reshape(7, 128, 2, C).transpose(0, 2, 1, 3).reshape(14 * 128, C)
            ye[: 14 * 128] = yp
            ye[14 * 128 : 15 * 128] = y_sing[0]
            ye[15 * 128 :] = y_sing[1]
            ye = ye.T  # [C, H]
            out[sel] += comb[sel, e][:, None] * ye[: len(sel)]
    return out


# revision 22
# speedup vs baseline: 1.1880x; 1.0096x over previous
"""Trainium2 Bass kernel for a top-2 MoE layer (T=2048, H=2048, I=1408, E=8).

Strategy: expert-parallel over 8 NeuronCores. The host dispatches tokens:
for each expert e it gathers the tokens routed to e (~480 of 2048, padded
to a shared capacity C sized to the busiest expert), so each core runs a
dense [C,H]x[2I,H]->silu*mul->[C,H] FFN for its expert — a 4x FLOP saving
over dense all-experts compute. The host then combines per-expert outputs
with the routing weights.

Device kernel (per core), transposed layout so no on-device transposes:
  stage 1: guT[2816, C] = w13 @ xT         (22 x 16 matmuls, K-tiles of 128)
  stage 2: actT[1408, C] = silu(gT) * uT   (ScalarE Silu + VectorE mul)
  stage 3: yT[2048, C] = w2 @ actT         (16 x 11 matmuls)
Matmuls in fp16 (fp32 accumulation in PSUM; rel-err ~5e-4).

Schedule details:
 - 36 warmup matmuls on a memset scratch tile bridge the initial input-DMA
   wait (~6.5us -> ~13.5us), so the PE's HAM clock gate is already 8/8
   when the real stream starts (cold matmuls run at half clock). The
   measured exec window opens at the framework preamble either way, so
   the warmups are free; they must end close to data arrival or a >3.4us
   PE idle re-throttles the clock gate.
 - Head transfers are staggered in consumption order across the two HWDGE
   trigger engines (Sync: x chunks, Scalar: m0 weight quarters) and the
   m=0 gate/up chains are interleaved in blocks of 4 k-tiles, halving the
   x-bandwidth the first chains demand while the 3MB head burst (shared
   HBM, all 8 cores at once) is still in flight.
 - Few, large DMA transfers (w13 g+u fused per m-block for m>=3, w2 in 4
   super-tiles, x in 4 chunks, y in pairs) cut trigger overhead.
 - psy (stage-3 PSUM) is allocated alongside psg (8 banks total) so the
   first stage-3 matmul doesn't wait on the stage-1 bank release; w2 gets
   3 buffers so its 3rd super-tile load isn't gated by a tile release.
 - Stage-1-only tile pools close before stage 3 is traced, so their
   release bookkeeping hides under the stage-3 matmul stream.
 - y returns in fp16 (adds ~2e-4 rel error, halves output DMA) and the
   last yT row block is computed as two half-width chains, each half
   DMA'd as soon as its copy lands -- the end-of-kernel queue drain that
   the measured window includes is halved.

Measured (8 cores, all-core NTFF profiling, max over cores): ~134.0us in
the chip's 2.4GHz PE state vs 134.7us for the previous baseline; the
matmul stream itself runs gapless at N/2.4GHz+2.5ns per matmul. Note the
shared trn2 package sometimes sits in a ~2.0GHz throttled state (253ns
per matmul) where any config measures ~156-158us; comparisons are only
valid within the same clock regime.
"""

import sys

if "/opt/trn_rl_repo" not in sys.path:
    sys.path.insert(0, "/opt/trn_rl_repo")

import os
import numpy as np
from contextlib import ExitStack

import concourse.bass as bass
import concourse.tile as tile
from concourse import bacc, mybir

T, H, I, E, K = 2048, 2048, 1408, 8, 2
CMAX = 512                   # max token capacity per expert per pass (PSUM bank)
HT = H // 128                # 16 K-tiles over H
IT = I // 128                # 11 K-tiles over I
BT = 2 * I // 128            # 22 row-blocks of guT

import ml_dtypes

MODE = os.environ.get("KERNEL_DTYPE", "f16")
if MODE == "bf16":
    DT = mybir.dt.bfloat16
    NP_DT = ml_dtypes.bfloat16
elif MODE == "f16":
    DT = mybir.dt.float16
    NP_DT = np.float16
else:
    DT = mybir.dt.float32r
    NP_DT = np.float32

# Warmup matmuls bridge the initial DMA wait (~6.5us -> ~13us) keeping the
# PE's HAM clock-gate warm; the measured exec window opens at the framework
# preamble regardless, so these are free for the metric.
N_WARM = int(os.environ.get("KERNEL_WARMUP_MMS", "36"))

_cache: dict = {}


def _build_nc(C):
    """Build + compile the per-core FFN program (same program on all cores)."""
    nc = bacc.Bacc("TRN2", target_bir_lowering=False, debug=False, num_devices=E)
    # x in 4 chunks of 4 k-tiles: chunk j, cols kk*C:(kk+1)*C = k-tile 4j+kk
    x_d = nc.dram_tensor("x_sb", [4, 128, 4 * C], DT, kind="ExternalInput")
    # w13 m=0..2 as separate gate/up halves so early transfers are small and
    # can be staggered to match the matmul stream's consumption order
    w13a_d = nc.dram_tensor("w13a_sb", [6, 128, HT * 128], DT, kind="ExternalInput")
    # w13 m=3..10 fused gate+up: [128, 2*HT*128] each
    w13b_d = nc.dram_tensor(
        "w13b_sb", [IT - 3, 128, 2 * HT * 128], DT, kind="ExternalInput"
    )
    # w2 in 4 super-tiles of 4 m-blocks: cols ((m%4)*IT + k)*128
    w2_d = nc.dram_tensor("w2_sb", [4, 128, 4 * IT * 128], DT, kind="ExternalInput")
    # y out in fp16 (quantization ~2e-4 rel, well under budget; halves the
    # output DMA bytes and the end-of-kernel queue drain): 7 pairs + 2 singles
    y_d = nc.dram_tensor("y_sb", [7, 128, 2 * C], DT, kind="ExternalOutput")
    y2_d = nc.dram_tensor("y2_sb", [2, 128, C], DT, kind="ExternalOutput")

    AF = mybir.ActivationFunctionType
    F32 = mybir.dt.float32
    CL = C // 2          # last-block split: first half columns
    CR = C - CL

    with tile.TileContext(nc) as tc, ExitStack() as ctx:
        # ---- warmup: keep the PE busy (and HAM warm) while inputs land ----
        with tc.tile_pool(name="warm", bufs=1) as wrm, tc.tile_pool(
            name="warmp", bufs=1, space=bass.MemorySpace.PSUM
        ) as wrmp:
            wt = wrm.tile([128, 256], DT, tag="wt")
            nc.gpsimd.memset(wt[:], 0.0)
            wps = wrmp.tile([128, 256], F32, tag="wps")
            for _ in range(N_WARM):
                nc.tensor.matmul(wps[:], wt[:, 0:128], wt[:], start=True, stop=True)

        # ---- stage 1+2 pools (closed before stage 3 so releases hide) ----
        act_t = []
        ap = ctx.enter_context(tc.tile_pool(name="act", bufs=1))
        wp2 = ctx.enter_context(tc.tile_pool(name="w2", bufs=3))
        # psy opened BEFORE psg so stage-3 PSUM tiles don't wait on the
        # stage-1 bank release (psg 5 + psy 3 = 8 banks coexist)
        psy = ctx.enter_context(
            tc.tile_pool(name="psy", bufs=3, space=bass.MemorySpace.PSUM)
        )
        with tc.tile_pool(name="x", bufs=1) as xp, tc.tile_pool(
            name="w13a", bufs=6
        ) as wpa, tc.tile_pool(name="w13b", bufs=3) as wpb, tc.tile_pool(
            name="tmp1", bufs=2
        ) as sp1, tc.tile_pool(
            name="psg", bufs=5, space=bass.MemorySpace.PSUM
        ) as psg:
            # Head transfers, staggered in consumption order. DMA queues
            # drain roughly FIFO per trigger engine, so issue order ==
            # arrival order; small first transfers start the stream early.
            x_t = []
            xc = []
            for j in range(4):
                xt = xp.tile([128, 4 * C], DT, tag=f"x{j}", name=f"x{j}")
                xc.append(xt)
            wa = [
                wpa.tile([128, HT * 128], DT, tag="w13a", name=f"w13a{i}")
                for i in range(6)
            ]  # g0,u0,g1,u1,g2,u2
            HB = HT * 128 // 2
            # sync engine: x chunks + m1 halves
            nc.sync.dma_start(xc[0][:], x_d.ap()[0])
            nc.sync.dma_start(xc[1][:], x_d.ap()[1])
            nc.sync.dma_start(xc[2][:], x_d.ap()[2])
            # scalar engine: m0 weight halves (quarter transfers) in the
            # order the interleaved g/u chains consume them, then x chunk 3
            nc.scalar.dma_start(wa[0][:, 0:HB], w13a_d.ap()[0][:, 0:HB])
            nc.scalar.dma_start(wa[1][:, 0:HB], w13a_d.ap()[1][:, 0:HB])
            nc.scalar.dma_start(wa[0][:, HB:], w13a_d.ap()[0][:, HB:])
            nc.scalar.dma_start(wa[1][:, HB:], w13a_d.ap()[1][:, HB:])
            nc.scalar.dma_start(xc[3][:], x_d.ap()[3])
            # m1 / m2 gate+up halves
            nc.sync.dma_start(wa[2][:], w13a_d.ap()[2])
            nc.sync.dma_start(wa[3][:], w13a_d.ap()[3])
            nc.scalar.dma_start(wa[4][:], w13a_d.ap()[4])
            nc.scalar.dma_start(wa[5][:], w13a_d.ap()[5])
            for j in range(4):
                for kk in range(4):
                    x_t.append(xc[j][:, kk * C : (kk + 1) * C])

            wgu = {}

            def _load_w13(m, eng):
                # m >= 3
                wb = wpb.tile([128, 2 * HT * 128], DT, tag="w13b", name=f"wgu{m}")
                eng.dma_start(wb[:], w13b_d.ap()[m - 3])
                wgu[m] = wb

            w2t = []

            def _load_w2(q):
                wt2 = wp2.tile([128, 4 * IT * 128], DT, tag="w2", name=f"w2q{q}")
                nc.sync.dma_start(wt2[:], w2_d.ap()[q])
                w2t.append(wt2)

            # stage 1+2: guT blocks
            for m in range(IT):
                if m < 3:
                    g_w = [wa[2 * m][:, k * 128 : (k + 1) * 128] for k in range(HT)]
                    u_w = [
                        wa[2 * m + 1][:, k * 128 : (k + 1) * 128] for k in range(HT)
                    ]
                else:
                    if m not in wgu:
                        _load_w13(m, nc.sync)
                    wb = wgu.pop(m)
                    g_w = [wb[:, k * 128 : (k + 1) * 128] for k in range(HT)]
                    u_w = [
                        wb[:, (HT + k) * 128 : (HT + k + 1) * 128] for k in range(HT)
                    ]
                g_ps = psg.tile([128, C], F32, tag="ps")
                u_ps = psg.tile([128, C], F32, tag="ps")
                if m == 0:
                    # interleave g/u in blocks of 4 k-tiles: halves the x
                    # delivery rate the first chains demand while the head
                    # DMA burst is still in flight
                    for kb in range(0, HT, 4):
                        for k in range(kb, kb + 4):
                            nc.tensor.matmul(
                                g_ps[:], g_w[k], x_t[k],
                                start=(k == 0), stop=(k == HT - 1),
                            )
                        for k in range(kb, kb + 4):
                            nc.tensor.matmul(
                                u_ps[:], u_w[k], x_t[k],
                                start=(k == 0), stop=(k == HT - 1),
                            )
                else:
                    for k in range(HT):
                        nc.tensor.matmul(
                            g_ps[:], g_w[k], x_t[k],
                            start=(k == 0), stop=(k == HT - 1),
                        )
                    for k in range(HT):
                        nc.tensor.matmul(
                            u_ps[:], u_w[k], x_t[k],
                            start=(k == 0), stop=(k == HT - 1),
                        )
                sg = sp1.tile([128, C], F32, tag="sg")
                nc.scalar.activation(sg[:], g_ps[:], AF.Silu)
                at = ap.tile([128, C], DT, tag=f"act{m}")
                nc.vector.tensor_mul(at[:], sg[:], u_ps[:])
                act_t.append(at)
                # prefetch: next-next w13 block + w2 super-tiles mid-stream
                if m + 2 < IT and m + 2 >= 3 and (m + 2) not in wgu:
                    _load_w13(m + 2, nc.sync if m % 2 else nc.scalar)
                if m == 4:
                    _load_w2(0)
                elif m == 6:
                    _load_w2(1)
                elif m == 8:
                    _load_w2(2)
                elif m == 10:
                    _load_w2(3)

        # ---- stage 3: yT row-blocks ----
        sp2 = ctx.enter_context(tc.tile_pool(name="tmp2", bufs=2))
        # half-width PSUM tiles for the split last chain; opened after psg
        # closed (reuses freed banks, long before they're needed)
        psyh = ctx.enter_context(
            tc.tile_pool(name="psyh", bufs=2, space=bass.MemorySpace.PSUM)
        )
        ysb = []
        for m in range(HT):
            qq = m // 4
            w2tile = w2t[qq]
            coff = (m % 4) * IT * 128

            if m < 14:
                if m % 2 == 0:
                    yt = sp2.tile([128, 2 * C], DT, tag="yout2")
                    ysb.append(yt)
                y_sb = ysb[m // 2]
                dst = y_sb[:, (m % 2) * C : (m % 2 + 1) * C]
            else:
                yt = sp2.tile([128, C], DT, tag="yout1")
                ysb.append(yt)
                y_sb = yt
                dst = y_sb[:]

            if m == HT - 1:
                # split the final chain so the tail drain is half-width,
                # and DMA each half out as soon as its copy lands
                y_psL = psyh.tile([128, CL], F32, tag="yh")
                y_psR = psyh.tile([128, CR], F32, tag="yh")
                for k in range(IT):
                    wsl = w2tile[:, coff + k * 128 : coff + (k + 1) * 128]
                    nc.tensor.matmul(
                        y_psL[:], wsl, act_t[k][:, 0:CL],
                        start=(k == 0), stop=(k == IT - 1),
                    )
                # DVE copy: ~170ns faster than ACT and the R copy is on the
                # kernel's critical path
                nc.vector.tensor_copy(y_sb[:, 0:CL], y_psL[:])
                nc.sync.dma_start(y2_d.ap()[1][:, 0:CL], y_sb[:, 0:CL])
                for k in range(IT):
                    wsl = w2tile[:, coff + k * 128 : coff + (k + 1) * 128]
                    nc.tensor.matmul(
                        y_psR[:], wsl, act_t[k][:, CL:C],
                        start=(k == 0), stop=(k == IT - 1),
                    )
                nc.vector.tensor_copy(y_sb[:, CL:C], y_psR[:])
                nc.sync.dma_start(y2_d.ap()[1][:, CL:C], y_sb[:, CL:C])
            else:
                y_ps = psy.tile([128, C], F32, tag="y")
                for k in range(IT):
                    nc.tensor.matmul(
                        y_ps[:], w2tile[:, coff + k * 128 : coff + (k + 1) * 128],
                        act_t[k][:],
                        start=(k == 0), stop=(k == IT - 1),
                    )
                nc.scalar.copy(dst, y_ps[:])
                if m < 14 and m % 2 == 1:
                    nc.sync.dma_start(y_d.ap()[m // 2], y_sb[:])
                elif m == 14:
                    nc.sync.dma_start(y2_d.ap()[0], y_sb[:])

    nc.compile()
    return nc


def _get_nc(C):
    if C not in _cache:
        _cache[C] = _build_nc(C)
    return _cache[C]


def _prep_weights(w13, w2):
    """Pre-tile weights into the SBUF layouts the kernel DMAs verbatim.

    w13 k-tile layout per row-block b (of 22): [128 rows, HT*128 cols] where
    col k*128+p holds w13[e, b*128+row, k*128+p] transposed so the 128
    partition dim is the contraction dim:
      w13_blk[e, b, p, k*128+c] = w13[e, b*128+c, k*128+p]
    Gate block m pairs with up block m+IT.
      w13a: m=0 gate, m=0 up (two separate transfers)
      w13b: m=1..10, gate||up fused into [128, 2*HT*128]
    w2 super-tile q holds m-blocks 4q..4q+3:
      w2_sb[e, q, p, ((m%4)*IT+k)*128+c] = w2[e, m*128+c, k*128+p]
    """
    w13_blk = (
        w13.reshape(E, BT, 128, HT, 128)
        .transpose(0, 1, 4, 3, 2)
        .astype(NP_DT)
        .reshape(E, BT, 128, HT * 128)
    )
    # m=0..2 as separate gate/up halves: [g0,u0,g1,u1,g2,u2]
    w13a = np.ascontiguousarray(
        w13_blk[:, [0, IT, 1, IT + 1, 2, IT + 2]]
    )  # [E, 6, 128, HT*128]
    gu = np.stack([w13_blk[:, 3:IT], w13_blk[:, IT + 3 :]], axis=2)
    # gu: [E, 8, 2, 128, HT*128] -> fuse (2, HT*128) cols per partition
    w13b = np.ascontiguousarray(gu.transpose(0, 1, 3, 2, 4)).reshape(
        E, IT - 3, 128, 2 * HT * 128
    )
    w2_blk = (
        w2.reshape(E, HT, 128, IT, 128)
        .transpose(0, 1, 4, 3, 2)
        .astype(NP_DT)
        .reshape(E, HT, 128, IT * 128)
    )  # [E, m, 128, IT*128]
    w2_sb = np.ascontiguousarray(
        w2_blk.reshape(E, 4, 4, 128, IT * 128).transpose(0, 1, 3, 2, 4)
    ).reshape(E, 4, 128, 4 * IT * 128)
    return w13a, w13b, w2_sb


def kernel(
    hidden_states,
    topk_weights,
    topk_ids,
    w13,
    w2,
    num_global_tokens=None,
    max_num_tokens_per_gpu=None,
):
    from concourse.bass_utils import run_bass_kernel_spmd

    hs = np.asarray(hidden_states, dtype=np.float32)
    tw = np.asarray(topk_weights, dtype=np.float32)
    ti = np.asarray(topk_ids)
    w13 = np.asarray(w13, dtype=np.float32)
    w2 = np.asarray(w2, dtype=np.float32)

    assert hs.shape == (T, H), hs.shape
    assert w13.shape == (E, 2 * I, H), w13.shape
    assert w2.shape == (E, H, I), w2.shape

    # per-(token, expert) combine weights: sum of topk weights routed to e
    comb = np.zeros((T, E), dtype=np.float32)
    for k in range(ti.shape[1]):
        col = ti[:, k]
        ok = (col >= 0) & (col < E)
        np.add.at(comb, (np.arange(T)[ok], col[ok]), tw[ok, k])

    idxs = [np.nonzero(comb[:, e])[0] for e in range(E)]
    need = max(len(ix) for ix in idxs)
    C = min(CMAX, max(64, need))
    nchunks = max(1, -(-need // C))

    w13a, w13b, w2_sb = _prep_weights(w13, w2)
    nc = _get_nc(C)

    trace = bool(os.environ.get("KERNEL_PROFILE"))
    out = np.zeros((T, H), dtype=np.float32)
    for chunk in range(nchunks):
        in_maps = []
        sels = []
        for e in range(E):
            sel = idxs[e][chunk * C : (chunk + 1) * C]
            xe = np.zeros((C, H), dtype=np.float32)
            xe[: len(sel)] = hs[sel]
            # x chunks: [4, 128, 4*C]; chunk j col block kk = k-tile 4j+kk
            x_sb = (
                np.ascontiguousarray(
                    xe.T.reshape(4, 4, 128, C).transpose(0, 2, 1, 3)
                )
                .astype(NP_DT, copy=False)
                .reshape(4, 128, 4 * C)
            )
            in_maps.append(
                {
                    "x_sb": x_sb,
                    "w13a_sb": w13a[e],
                    "w13b_sb": w13b[e],
                    "w2_sb": w2_sb[e],
                }
            )
            sels.append(sel)
        if trace:
            try:
                res = run_bass_kernel_spmd(nc, in_maps, list(range(E)), trace=True)
                if res.exec_time_ns is not None:
                    print(f"HW exec time: {res.exec_time_ns} ns")
            except Exception:
                res = run_bass_kernel_spmd(nc, in_maps, list(range(E)))
        else:
            res = run_bass_kernel_spmd(nc, in_maps, list(range(E)))
        for e in range(E):
            sel = sels[e]
            if len(sel) == 0:
                continue
            y_pair = np.asarray(res.results[e]["y_sb"], dtype=np.float32)
            y_sing = np.asarray(res.results[e]["y2_sb"], dtype=np.float32)
            # decode: pairs hold m=2j (cols 0:C) and m=2j+1 (cols C:2C)
            ye = np.empty((H, C), dtype=np.float32)
            yp = y_pair.reshape(7, 128, 2, C).transpose(0, 2, 1, 3).reshape(14 * 128, C)
            ye[: 14 * 128] = yp
            ye[14 * 128 : 15 * 128] = y_sing[0]
            ye[15 * 128 :] = y_sing[1]
            ye = ye.T  # [C, H]
            out[sel] += comb[sel, e][:, None] * ye[: len(sel)]
    return out
